# revision 3
# baseline (speedup 1.0000x reference)
"""Trainium2 Bass kernel for relational GNN message passing (BlockDecomposition).

v2 design ("one-hop gather + descriptor-free pass-2"):
  - Partition output nodes across 8 cores (12500 each), split into 25
    windows of 512 targets. Edges sorted by (window, src-range, relation)
    with group capacities shared across cores (SPMD single program).
  - x rows gathered per edge straight from DRAM (Q7 dma_gather, 4 ops per
    window, int16 indices via 4 src ranges) -> row tiles; PE-transposed
    into xT slabs.
  - Pass 1: per (window, relation, range-subrun) matmul with stationary
    W_r streaming xT columns -> msgsT [feat, 512-slot] PSUM sections;
    cast to SBUF; PE-transpose back per 128-slot chunk; Act-engine cast
    applies the per-edge weight (per-partition scale) -> msgs slab.
  - Pass 2 (descriptor-free): per window accumulate
    out^T[feat, 512] += msgs_chunk^T @ Stilde_chunk on PE, where
    Stilde[e, t] is a host-built one-hot (bf16) streamed from DRAM.
    Transpose out^T, write out rows. No second gather, no DVE S-builds.
All structure sizes are cap-based (max over cores) so one instruction
stream serves all 8 cores.
"""

import os
import sys

sys.path.insert(0, "/opt/trn_rl_repo")

import numpy as np
import ml_dtypes

_PATCHED = False


def _patch_tile_drain():
    """This container's walrus accepts at most one sync-wait per instruction,
    but TileContext's kernel-tail attaches every outstanding DMA-lane wait to
    a single Drain ("Too many sync wait commands"). Spread the waits across
    individual SP NOPs before the drain."""
    global _PATCHED
    if _PATCHED:
        return
    _PATCHED = True
    import concourse.mybir as mybir
    import concourse.tile as tile_mod
    from bass_rust import ScopedClock

    def _drain_and_barrier(self, tick_clock, wait_clock):
        nc = self.nc
        collector = nc.sync.nop(nofuse=True, hint="drain_waits")
        wait_clock.add_sem_waits(
            collector.ins, ScopedClock({None: tick_clock.global_clock})
        )
        si = collector.ins.sync_info
        waits = list(si.on_wait) if si and si.on_wait else []
        if len(waits) > 1:
            si.on_wait = waits[:1]
            for wv in waits[1:]:
                n2 = nc.sync.nop(nofuse=True, hint="drain_waits")
                n2.ins.sync_info = mybir.SyncInfo(on_wait=[wv], on_update=[])
        nc.sync.drain()
        nc.all_engine_barrier()
        assert self.sems is not None
        popped = nc._tile_sem_poison_stack.pop()
        assert popped is self._sem_poison
        nc.clear_and_free_semaphores(list(self.sems.allocated().values()))
        nc.all_engine_barrier()

    tile_mod.TileContext._drain_and_barrier = _drain_and_barrier


# ---------------- problem constants (hardcoded) ----------------
N_NODES = 100000
D = 128
R = 64
NB = 8
BS = 16
N_EDGES = 500000
NCORES = 8
P = 128
NT = N_NODES // NCORES          # 12500 targets per core
WT = 1024                       # targets per window
SECW = 512                      # pass-1 section / PSUM width
NWIN = (NT + WT - 1) // WT      # 13 (last window 212 real targets)
OUT_ROWS = NWIN * WT            # 12800 padded output rows
RANGE = 32768
NRANGE = 4                      # src ranges: [0,32768,65536,98304,100000)
RANGE_LO = [0, 32768, 65536, 98304]
RANGE_HI = [32768, 65536, 98304, 100000]

TRACE = os.environ.get("GNN_TRACE", "0") == "1"

bf16 = ml_dtypes.bfloat16


def _idx_image(lst):
    """int16 index list (len % 128 == 0) -> SBUF image [128, len//16],
    entry i at (i%16, i//16), replicated across the 8 16-partition bands."""
    lst = np.asarray(lst, dtype=np.int16)
    n = len(lst)
    assert n % 128 == 0 and n > 0
    a = lst.reshape(n // 16, 16).T          # [16, n//16]
    return np.tile(a, (8, 1))               # [128, n//16]


def _ceil(a, b):
    return -(-a // b)


def _round_up(a, b):
    return _ceil(a, b) * b


def _preprocess(x, blocks, edge_weights, source, target, edge_type):
    src = np.asarray(source).astype(np.int64)
    tgt = np.asarray(target).astype(np.int64)
    rel = np.asarray(edge_type).astype(np.int64)
    w = np.asarray(edge_weights).astype(np.float32)

    s2 = np.concatenate([src, tgt])
    t2 = np.concatenate([tgt, src])
    r2 = np.concatenate([rel, rel])
    w2 = np.concatenate([w, w])

    owner = t2 // NT
    tloc = t2 - owner * NT

    NKEY = NWIN * NRANGE * R
    per_core = []
    counts = np.zeros((NCORES, NKEY), np.int64)
    for c in range(NCORES):
        m = owner == c
        cs, ct, cr, cw = s2[m], tloc[m], r2[m], w2[m]
        win = ct // WT
        rng = cs // RANGE
        key = (win * NRANGE + rng) * R + cr
        order = np.argsort(key, kind="stable")
        cs, ct, cr, cw, key = cs[order], ct[order], cr[order], cw[order], key[order]
        counts[c] = np.bincount(key, minlength=NKEY)
        per_core.append((cs, ct, cr, cw, key))
    cap = counts.max(axis=0)                 # [NKEY]

    # ---- shared slot layout ----
    # span (win, rng): groups rel=0..63 consecutive; span padded to 128;
    # window padded to 512 by extending its LAST nonempty span.
    group_base = np.zeros(NKEY, np.int64)    # global slot base per group
    span_info = []                           # per (win, rng): (slot0, len, rng)
    win_info = []                            # per win: (slot0, nchunk, nsec)
    runs_by_winsec = {}                      # (win, sec) -> list of (col, n, rel)
    pos = 0
    for wv in range(NWIN):
        w_slot0 = pos
        spans_raw = []
        for q in range(NRANGE):
            caps = cap[(wv * NRANGE + q) * R:(wv * NRANGE + q) * R + R]
            slen = int(caps.sum())
            spans_raw.append(slen)
        # window total with span padding
        padded = [_round_up(s, P) if s else 0 for s in spans_raw]
        wlen = sum(padded)
        wpad_target = _round_up(max(wlen, WT), WT)
        extra = wpad_target - wlen
        # absorb window pad into last nonempty span
        last_ne = max((q for q in range(NRANGE) if padded[q] > 0), default=0)
        if extra and padded[last_ne] == 0:
            padded[last_ne] = extra
            spans_raw[last_ne] = 0
            extra = 0
        padded[last_ne] += extra
        # lay out spans and groups, build runs
        runs = []                            # (win-local col, n, rel)
        for q in range(NRANGE):
            if padded[q] == 0:
                continue
            span0 = pos - w_slot0            # win-local
            cur = span0
            caps_q = cap[(wv * NRANGE + q) * R:(wv * NRANGE + q) * R + R]
            last_rel = 0
            for r in range(R):
                cnum = int(caps_q[r])
                group_base[(wv * NRANGE + q) * R + r] = (w_slot0 + cur)
                if cnum:
                    runs.append([cur, cnum, r])
                    last_rel = len(runs) - 1
                cur += cnum
            tail = padded[q] - (cur - span0)
            if tail:
                if runs and runs[last_rel][0] + runs[last_rel][1] == cur:
                    runs[last_rel][1] += tail
                else:
                    runs.append([cur, tail, 0])
                cur += tail
            span_info.append((w_slot0 + span0, padded[q], q))
            pos += padded[q]
        nchunk = (pos - w_slot0) // P
        nsec = (pos - w_slot0) // SECW
        win_info.append((w_slot0, nchunk, nsec))
        # split runs at section boundaries, group by section
        for (col, n, r) in runs:
            while n > 0:
                sec = col // SECW
                take = min(n, (sec + 1) * SECW - col)
                runs_by_winsec.setdefault((wv, sec), []).append(
                    (col - sec * SECW, take, r))
                col += take
                n -= take
    TOTSLOTS = pos
    NCHUNK = TOTSLOTS // P
    # every section must be fully covered by pass-1 runs
    for (wv, sec), rl in runs_by_winsec.items():
        assert sum(n for (_c, n, _r) in rl) == SECW, (wv, sec, rl)

    # ---- per-core data images ----
    ximgs, wimgs, sarrs = [], [], []
    for c in range(NCORES):
        cs, ct, cr, cw, key = per_core[c]
        # rank within group (key sorted): position - first index of that key
        kcnt = np.bincount(key, minlength=NKEY)
        first_idx = np.concatenate([[0], np.cumsum(kcnt)])[:-1]
        rank = np.arange(len(key)) - first_idx[key]
        slot = group_base[key] + rank
        # x idx values (int16, range-relative)
        vals = np.zeros(TOTSLOTS, np.int64)
        qq = key // R % NRANGE
        vals[slot] = cs - qq * RANGE
        # wrap per span
        ximg = np.zeros((P, TOTSLOTS // 16), np.int16)
        for (s0, slen, q) in span_info:
            ximg[:, s0 // 16:(s0 + slen) // 16] = _idx_image(vals[s0:s0 + slen])
        ximgs.append(ximg)
        # w image [128, NCHUNK] f32
        wimg = np.zeros((P, NCHUNK), np.float32)
        wimg[slot % P, slot // P] = cw
        wimgs.append(wimg)
        # Stilde [128, NCHUNK*WT] bf16: one-hot of window-local target
        sarr = np.zeros((P, NCHUNK * WT), ml_dtypes.float8_e4m3)
        tcol = ct - (ct // WT) * WT
        sarr[slot % P, (slot // P) * WT + tcol] = 1.0
        sarrs.append(sarr)

    # ---- shared weights ----
    wd = np.zeros((P, R * P), dtype=bf16)
    blk = np.asarray(blocks, dtype=np.float32)
    for r in range(R):
        for b in range(NB):
            wd[b * BS:(b + 1) * BS, r * P + b * BS:r * P + (b + 1) * BS] = \
                blk[r, b].astype(bf16)
    xbf = np.asarray(x, dtype=np.float32).astype(bf16)
    ident_bf = np.eye(P, dtype=np.float32).astype(bf16)
    ident_f32 = np.eye(P, dtype=np.float32)

    shared = {
        "span_info": span_info,
        "win_info": win_info,
        "runs": runs_by_winsec,
        "TOTSLOTS": TOTSLOTS,
        "NCHUNK": NCHUNK,
        "MAXWCH": max(wi[1] for wi in win_info),
    }
    in_maps = []
    for c in range(NCORES):
        in_maps.append({
            "xbf": xbf,
            "wd": wd,
            "identb": ident_bf,
            "identf": ident_f32,
            "ximg": ximgs[c],
            "wimg": wimgs[c],
            "stil": sarrs[c],
        })
    return shared, in_maps


def _build_nc(shared):
    _patch_tile_drain()
    import concourse.bacc as bacc
    import concourse.mybir as mybir
    from concourse.tile import TileContext

    f32 = mybir.dt.float32
    bf = mybir.dt.bfloat16
    f8 = mybir.dt.float8e4
    i16 = mybir.dt.int16

    span_info = shared["span_info"]
    win_info = shared["win_info"]
    runs = shared["runs"]
    TOTSLOTS = shared["TOTSLOTS"]
    NCHUNK = shared["NCHUNK"]
    MAXWCH = shared["MAXWCH"]

    nc = bacc.Bacc("TRN2", target_bir_lowering=False, debug=False)
    xbf_d = nc.dram_tensor("xbf", [N_NODES, D], bf, kind="ExternalInput")
    wd_d = nc.dram_tensor("wd", [P, R * P], bf, kind="ExternalInput")
    identb_d = nc.dram_tensor("identb", [P, P], bf, kind="ExternalInput")
    identf_d = nc.dram_tensor("identf", [P, P], f32, kind="ExternalInput")
    ximg_d = nc.dram_tensor("ximg", [P, TOTSLOTS // 16], i16, kind="ExternalInput")
    wimg_d = nc.dram_tensor("wimg", [P, NCHUNK], f32, kind="ExternalInput")
    stil_d = nc.dram_tensor("stil", [P, NCHUNK * WT], f8, kind="ExternalInput")
    out_d = nc.dram_tensor("out", [OUT_ROWS, D], f32, kind="ExternalOutput")

    # group spans by window for the gather loop
    spans_by_win = [[] for _ in range(NWIN)]
    for (s0, slen, q) in span_info:
        # find window: s0 falls inside exactly one window
        for wv in range(NWIN):
            w0, nch, _ = win_info[wv]
            if w0 <= s0 < w0 + nch * P:
                spans_by_win[wv].append((s0, slen, q))
                break

    with TileContext(nc) as tc:
        with (
            tc.tile_pool(name="cp", bufs=1) as cp,
            tc.tile_pool(name="gp", bufs=2) as gp,
            tc.tile_pool(name="xtp", bufs=2) as xtp,
            tc.tile_pool(name="mts", bufs=3) as mts,
            tc.tile_pool(name="msl", bufs=1) as mslp,
            tc.tile_pool(name="stp", bufs=3) as stp,
            tc.tile_pool(name="op", bufs=2) as op_pool,
            tc.tile_pool(name="ob", bufs=3) as ob_pool,
            tc.tile_pool(name="ps_pt", bufs=2, space="PSUM") as ps_pt,
            tc.tile_pool(name="ps_pm", bufs=2, space="PSUM") as ps_pm,
            tc.tile_pool(name="ps_mt", bufs=2, space="PSUM") as ps_mt,
            tc.tile_pool(name="ps_ot", bufs=1, space="PSUM") as ps_ot,
        ):
            wd_t = cp.tile([P, R * P], bf)
            identb_t = cp.tile([P, P], bf)
            identf_t = cp.tile([P, P], f32)
            ximg_t = cp.tile([P, TOTSLOTS // 16], i16)
            wimg_t = cp.tile([P, NCHUNK], f32)
            nc.sync.dma_start(out=wd_t[:], in_=wd_d[:])
            nc.sync.dma_start(out=identb_t[:], in_=identb_d[:])
            nc.sync.dma_start(out=identf_t[:], in_=identf_d[:])
            nc.sync.dma_start(out=ximg_t[:], in_=ximg_d[:])
            nc.sync.dma_start(out=wimg_t[:], in_=wimg_d[:])

            for wv in range(NWIN):
                w_slot0, wch, wsec = win_info[wv]
                # ---- gather x rows for this window ----
                G = gp.tile([P, MAXWCH, D], bf, tag="G")
                for (s0, slen, q) in spans_by_win[wv]:
                    lc0 = (s0 - w_slot0) // P
                    nc.gpsimd.dma_gather(
                        out_ap=G[:, lc0:lc0 + slen // P, :],
                        in_ap=xbf_d[RANGE_LO[q]:RANGE_HI[q], :],
                        idxs_ap=ximg_t[:, s0 // 16:(s0 + slen) // 16],
                        num_idxs=slen, num_idxs_reg=slen, elem_size=D,
                        single_packet=False,
                    )
                # ---- transpose rows -> xT slab ----
                xT = xtp.tile([P, MAXWCH * P], bf, tag="xT")
                for j in range(wch):
                    pT = ps_pt.tile([P, P], bf, tag="pT")
                    nc.tensor.transpose(out=pT[:], in_=G[:, j, :],
                                        identity=identb_t[:])
                    nc.vector.tensor_copy(out=xT[:, j * P:(j + 1) * P],
                                          in_=pT[:])
                # ---- pass 1: msgsT sections, transpose back, w-scale ----
                M = mslp.tile([P, MAXWCH, D], bf, tag="M")
                for sec in range(wsec):
                    mT = ps_mt.tile([P, SECW], f32, tag="mT")
                    for (col, n, r) in runs.get((wv, sec), []):
                        nc.tensor.matmul(
                            out=mT[:, col:col + n],
                            lhsT=wd_t[:, r * P:(r + 1) * P],
                            rhs=xT[:, sec * SECW + col:sec * SECW + col + n],
                            start=True, stop=True, skip_group_check=True,
                        )
                    mTs = mts.tile([P, SECW], bf, tag="mTs")
                    nc.vector.tensor_copy(out=mTs[:], in_=mT[:])
                    for qq in range(SECW // P):
                        j = sec * (SECW // P) + qq
                        pM = ps_pm.tile([P, P], bf, tag="pM")
                        nc.tensor.transpose(out=pM[:], in_=mTs[:, qq * P:(qq + 1) * P],
                                            identity=identb_t[:])
                        gj = w_slot0 // P + j
                        nc.scalar.activation(
                            out=M[:, j, :], in_=pM[:],
                            func=mybir.ActivationFunctionType.Copy,
                            scale=wimg_t[:, gj:gj + 1])
                # ---- pass 2: out^T accumulation ----
                oT = ps_ot.tile([P, WT], f32, tag="oT")
                SG = 4  # S chunks per load
                for jg in range(0, wch, SG):
                    jn = min(SG, wch - jg)
                    St = stp.tile([P, SG * WT], f8, tag="St")
                    gc0 = w_slot0 // P + jg
                    nc.sync.dma_start(
                        out=St[:, :jn * WT],
                        in_=stil_d[:, gc0 * WT:(gc0 + jn) * WT])
                    for j in range(jg, jg + jn):
                        for hh in range(WT // SECW):
                            nc.tensor.matmul(
                                out=oT[:, hh * SECW:(hh + 1) * SECW],
                                lhsT=M[:, j, :],
                                rhs=St[:, (j - jg) * WT + hh * SECW:
                                        (j - jg) * WT + (hh + 1) * SECW],
                                start=(j == 0), stop=(j == wch - 1),
                                skip_group_check=True,
                            )
                # ---- transpose out^T -> out rows ----
                oTs = op_pool.tile([P, WT], bf, tag="oTs")
                nc.vector.tensor_copy(out=oTs[:], in_=oT[:])
                for qq in range(WT // P):
                    pO = ps_pt.tile([P, P], bf, tag="pT")
                    nc.tensor.transpose(out=pO[:], in_=oTs[:, qq * P:(qq + 1) * P],
                                        identity=identb_t[:])
                    ob = ob_pool.tile([P, P], f32, tag="ob")
                    nc.scalar.activation(
                        out=ob[:], in_=pO[:],
                        func=mybir.ActivationFunctionType.Copy)
                    nc.sync.dma_start(
                        out=out_d[wv * WT + qq * P:wv * WT + (qq + 1) * P, :],
                        in_=ob[:])
    nc.finalize()
    return nc


def kernel(x, blocks, edge_weights, source, target, edge_type):
    from concourse import bass_utils

    shared, in_maps = _preprocess(x, blocks, edge_weights, source, target,
                                  edge_type)
    nc = _build_nc(shared)
    res = bass_utils.run_bass_kernel_spmd(
        nc, in_maps, core_ids=list(range(NCORES)), trace=TRACE,
    )
    out = np.concatenate([res.results[c]["out"][:NT] for c in range(NCORES)],
                         axis=0)
    if TRACE:
        kernel.last_exec_ns = res.exec_time_ns
    return out.astype(np.float32)


kernel.last_exec_ns = None
